# revision 14
# baseline (speedup 1.0000x reference)
"""LIF neuron kernel for Trainium2, 8-core SPMD (feature-sharded).

Reference semantics per timestep t (fp32, TAU=0.5):
    u   = 0.5*m + x_t          # leaky integrate
    s   = (u >= thresh)        # fire (output, 1.0/0.0)
    m'  = u * (u < thresh)     # hard reset

Shifted state w = u - th turns the step into

    w' = 0.5*select(w < 0, w, -th) + (x' - 0.5*th)

and this recurrence is exactly scale-invariant (halve, compare-to-0,
add), so the device works in a scaled integer domain W = w/scale:
the additive input Y = round((x - 0.5*th)/scale) ships as INT16
(halving input DMA bytes), converted to f32 on the fly by the DVE.

Features are sorted by thresh and grouped 4-per-partition (8 cores x
128 partitions x 4 = 4096), so the only remaining use of th on device
-- the post-spike reset arm -- becomes the per-partition quartet
midpoint, passed as the custom op's [P,1] s0 scalar.  The whole
timestep is then ONE custom DVE op per column half:

    W' = imm2 * select(W < 0, W, s0) + in1   (imm2=0.5, s0=-th_mid/scale)

Approximations (vs the fp32 reference, measured on the real inputs):
  - int16 input quantization (|err| <= scale/2 ~ 8.9e-5)
  - quartet-midpoint reset value (|dev| ~ 1e-5, decays 2x per step)
together flip ~500 of 26.2M spikes -> rel err ~9e-3 (tolerance 2e-2).
The spike compare W >= 0 uses the true per-feature threshold via Y.

Per-core layout: partition p = thresh-sorted quartet, free f = j*64+b
(quartet member j, batch b).  x is host-permuted to [128, 100, 256]
int16 per core so chunk DMAs move 5KB contiguous runs per partition.

Schedule (one core): per timestep the chain is TWO half-width custom
ops (cols 0:128 / 128:256) so each op's semaphore round-trip hides
under the other half's execution (387 ns/step vs 422 single-op).
Y arrives in chunks of [3,3,4,10*8,8,2] steps on the SP queue (a
just-in-time head cascade: each piece's 900ns-delayed DMA semaphore
lands exactly at the chain's arrival, within the 625ns/DMA HWDGE
budget; small tail chunks shorten the drain); spikes are extracted
in batched is_ge ops (uint8 out):
10-step groups on Pool, the final 8+2 on DVE right after the chain
ends; spike out-DMAs go through the Activation engine's DGE (tail
outs through SP, idle by then) so they never head-of-line-block the
Y in-DMAs.
Spikes return as uint8 and are unpacked/unpermuted to f32 on host.
"""

import numpy as np

import concourse.bacc as bacc
import concourse.mybir as mybir
from concourse import tile
from concourse.bass_utils import run_bass_kernel_spmd

B, T, N = 64, 100, 4096
NCORES = 8
NPC = N // NCORES         # 512 features per core
G = 4                     # quartet: features per partition
P = NPC // G              # 128 partitions
F = G * B                 # 256 free elems (quartet member x batch)
H = F // 2                # half-width for the interleaved chain streams

# pipeline structure (tuned against the TimelineSim cost model)
CHUNKS = [3, 3, 4] + [10] * 8 + [8, 2]          # Y in-DMA chunking
SPIKE_GROUPS = [(i * 10, 10, "pool") for i in range(8)] + [
    (80, 5, "pool"),
    (85, 5, "pool"),
    (90, 8, "dve"),
    (98, 2, "dve"),
]
XBUFS = {10: 6, 8: 3, 4: 4, 3: 3, 2: 3}
SBUFS = 8

_F32 = mybir.dt.float32
_I16 = mybir.dt.int16
_U8 = mybir.dt.uint8
_ALU = mybir.AluOpType

# ---------------------------------------------------------------- custom op --

_LIF_OP = None


def _register_lif_op():
    """Register the fused step op imm2*select(w<0, w, s0) + in1 at runtime."""
    global _LIF_OP
    if _LIF_OP is not None:
        return _LIF_OP
    from concourse.dve_spec import C0, C2, Spec, Src0, Src1, Zero, select, lower
    from concourse.dve_uop import DveOpSpec
    from concourse import dve_ops as dom

    name = "LIF_STEP_ANT"
    for op in dom.OPS:
        if op.name == name:
            _LIF_OP = op
            return op

    spec = Spec(
        body=C2 * select(Src0 < Zero, Src0, C0) + Src1,
        reference=lambda in0, in1, s0, s1, imm2: (
            np.float32(imm2) * np.where(in0 < 0, in0, s0).astype(np.float32) + in1
        ).astype(np.float32),
    )
    shas = {}
    for ver in ("v3", "v4"):
        try:
            tmp = DveOpSpec(name=name, opcode=None, uops=lower(spec, ver=ver), rd1_en=True)
            shas[ver] = tmp.sha(ver)
        except Exception:
            pass
    op = dom.DveOp(name, spec, subdim=False, uops_sha=shas)
    dom.OPS.append(op)
    dom._SUB_OPCODE_FOR_NAME[name] = dom._CUSTOM_DVE_ROW_BASE + len(dom.OPS) - 1
    dom.CUSTOM_DVE_SPECS[name] = spec
    _LIF_OP = op
    return op


# ------------------------------------------------------------------ program --

_NC_CACHE = {}


def _build_bass():
    if "nc" in _NC_CACHE:
        return _NC_CACHE["nc"]
    from collections import Counter

    lif_op = _register_lif_op()
    ccount = Counter(CHUNKS)
    gcount = Counter(gl for _, gl, _ in SPIKE_GROUPS)

    nc = bacc.Bacc("TRN2", name="lif_kernel")
    yt = nc.dram_tensor("yt", [P, T, F], _I16, kind="ExternalInput")
    nth = nc.dram_tensor("nth", [P, 2], _F32, kind="ExternalInput")
    spk = nc.dram_tensor("spk", [P, T, F], _U8, kind="ExternalOutput")

    with tile.TileContext(nc) as tc:
        with (
            tc.tile_pool(name="const", bufs=1) as cpool,
            tc.tile_pool(name="xin", bufs=1) as xpool,
            tc.tile_pool(name="sout", bufs=1) as spool,
        ):
            # nth col 0 = -th_mid/scale (reset arm), col 1 = -0.5*th_mid/scale
            nth_t = cpool.tile([P, 2], _F32)
            # w history: slot t+1 holds W after step t (slot 0 unused)
            w_all = cpool.tile([P, T + 1, F], _F32)

            gi = 0
            first = True
            t0c = 0
            for tchk in CHUNKS:
                x_tile = xpool.tile(
                    [P, tchk, F], _I16, tag=f"x{tchk}",
                    bufs=min(XBUFS[tchk], ccount[tchk]),
                )
                nc.sync.dma_start(x_tile[:], yt[:, t0c:t0c + tchk, :])
                if first:
                    # nth rides Pool's SWDGE so its descriptor gen never
                    # occupies the shared HWDGE slot the x chunks need
                    first = False
                    nc.gpsimd.dma_start(nth_t[:], nth[:])
                for tl in range(tchk):
                    t = t0c + tl
                    if t == 0:
                        # W0 = 0 makes the select trivial: 2x_2p tensor_scalar
                        # ops replace the two custom ops; issued as halves so
                        # the successors' sem round-trips overlap execution
                        for a, b in ((0, H), (H, F)):
                            nc.vector.tensor_scalar(
                                out=w_all[:, 1, a:b], in0=x_tile[:, 0, a:b],
                                scalar1=1.0, scalar2=nth_t[:, 1:2],
                                op0=_ALU.mult, op1=_ALU.add,
                            )
                        continue
                    nc.vector._custom_dve(
                        lif_op, out=w_all[:, t + 1, 0:H], in0=w_all[:, t, 0:H],
                        in1=x_tile[:, tl, 0:H], s0=nth_t[:, 0:1], imm2=0.5,
                    )
                    nc.vector._custom_dve(
                        lif_op, out=w_all[:, t + 1, H:F], in0=w_all[:, t, H:F],
                        in1=x_tile[:, tl, H:F], s0=nth_t[:, 0:1], imm2=0.5,
                    )
                t0c += tchk
                while gi < len(SPIKE_GROUPS) and SPIKE_GROUPS[gi][0] + SPIKE_GROUPS[gi][1] <= t0c:
                    g0, gl, eng = SPIKE_GROUPS[gi]
                    s_tile = spool.tile(
                        [P, gl, F], _U8, tag=f"s{gl}",
                        bufs=min((SBUFS if gl >= 10 else 3), gcount[gl]),
                    )
                    wsl = w_all[:, g0 + 1:g0 + gl + 1, :]
                    if eng == "dve":
                        if gl == 2:
                            # final group: halves hide the post-chain sem gap
                            for a, b in ((0, H), (H, F)):
                                nc.vector.tensor_scalar(
                                    out=s_tile[:, :, a:b],
                                    in0=w_all[:, g0 + 1:g0 + gl + 1, a:b],
                                    scalar1=0.0, scalar2=None, op0=_ALU.is_ge,
                                )
                        else:
                            nc.vector.tensor_scalar(
                                out=s_tile[:], in0=wsl, scalar1=0.0, scalar2=None,
                                op0=_ALU.is_ge,
                            )
                        # tail outs ride the (idle by now) SP queue
                        nc.sync.dma_start(spk[:, g0:g0 + gl, :], s_tile[:])
                    else:
                        nc.gpsimd.tensor_scalar(
                            out=s_tile[:], in0=wsl, scalar1=0.0, scalar2=None,
                            op0=_ALU.is_ge,
                        )
                        nc.scalar.dma_start(spk[:, g0:g0 + gl, :], s_tile[:])
                    gi += 1

    nc.finalize()
    _NC_CACHE["nc"] = nc
    return nc


# -------------------------------------------------------------------- entry --

def _prep_inputs(x, thresh):
    """Host-side permute/shift/quantize: returns (in_maps, order)."""
    x = np.ascontiguousarray(x, dtype=np.float32)
    thresh = np.ascontiguousarray(thresh, dtype=np.float32)
    order = np.argsort(thresh, kind="stable")
    in_maps = []
    for c in range(NCORES):
        feat = order[c * NPC:(c + 1) * NPC]
        th_c = thresh[feat]                           # sorted within core
        quart = th_c.reshape(P, G)
        th_mid = (quart[:, 0] + quart[:, -1]) * np.float32(0.5)
        ys = x[:, :, feat] - np.float32(0.5) * th_c[None, None, :]
        scale = np.float32(max(np.abs(ys).max() / 32000.0, 1e-30))
        yq = np.round(ys / scale).astype(np.int16)
        nth0 = (-th_mid / scale).astype(np.float32)
        nth = np.ascontiguousarray(
            np.stack([nth0, np.float32(0.5) * nth0], axis=1)
        )
        # [b, t, p*G+j] -> [p, t, j*B+b]
        yc = (
            yq.reshape(B, T, P, G)
            .transpose(2, 1, 3, 0)
            .reshape(P, T, F)
        )
        in_maps.append({"yt": np.ascontiguousarray(yc), "nth": nth})
    return in_maps, order


def _run(x, thresh, trace=False):
    nc = _build_bass()
    in_maps, order = _prep_inputs(x, thresh)
    res = run_bass_kernel_spmd(
        nc, in_maps, core_ids=list(range(NCORES)), trace=trace
    )
    out = np.empty((B, T, N), dtype=np.float32)
    for c in range(NCORES):
        feat = order[c * NPC:(c + 1) * NPC]
        s = np.asarray(res.results[c]["spk"])          # [P, T, F] u8
        vals = (
            s.reshape(P, T, G, B)
            .transpose(3, 1, 0, 2)
            .reshape(B, T, NPC)
        )
        out[:, :, feat] = vals.astype(np.float32)
    return out, res


def kernel(x, thresh):
    out, _ = _run(x, thresh, trace=False)
    return out


# revision 16
# speedup vs baseline: 1.0056x; 1.0056x over previous
"""LIF neuron kernel for Trainium2, 8-core SPMD (feature-sharded).

Reference semantics per timestep t (fp32, TAU=0.5):
    u   = 0.5*m + x_t          # leaky integrate
    s   = (u >= thresh)        # fire (output, 1.0/0.0)
    m'  = u * (u < thresh)     # hard reset

Shifted state w = u - th turns the step into

    w' = 0.5*select(w < 0, w, -th) + (x' - 0.5*th)

and this recurrence is exactly scale-invariant (halve, compare-to-0,
add), so the device works in a scaled integer domain W = w/scale:
the additive input Y = round((x - 0.5*th)/scale) ships as INT16
(halving input DMA bytes), converted to f32 on the fly by the DVE.

Features are sorted by thresh and grouped 4-per-partition (8 cores x
128 partitions x 4 = 4096), so the only remaining use of th on device
-- the post-spike reset arm -- becomes the per-partition quartet
midpoint, passed as the custom op's [P,1] s0 scalar.  The whole
timestep is then ONE custom DVE op per column half:

    W' = imm2 * select(W < 0, W, s0) + in1   (imm2=0.5, s0=-th_mid/scale)

Approximations (vs the fp32 reference, measured on the real inputs):
  - int16 input quantization (|err| <= scale/2 ~ 8.9e-5)
  - quartet-midpoint reset value (|dev| ~ 1e-5, decays 2x per step)
together flip ~500 of 26.2M spikes -> rel err ~9e-3 (tolerance 2e-2).
The spike compare W >= 0 uses the true per-feature threshold via Y.

Per-core layout: partition p = thresh-sorted quartet, free f = j*64+b
(quartet member j, batch b).  x is host-permuted to [128, 100, 256]
int16 per core so chunk DMAs move 5KB contiguous runs per partition.

Schedule (one core): per timestep the chain is TWO half-width custom
ops (cols 0:128 / 128:256) so each op's semaphore round-trip hides
under the other half's execution (387 ns/step vs 422 single-op).
Y arrives in chunks of [3,3,4,10*8,8,2] steps on the SP queue (a
just-in-time head cascade: each piece's 900ns-delayed DMA semaphore
lands exactly at the chain's arrival, within the 625ns/DMA HWDGE
budget; small tail chunks shorten the drain); spikes are extracted
in batched is_ge ops (uint8 out):
10-step groups on Pool, the final 8+2 on DVE right after the chain
ends; spike out-DMAs go through the Activation engine's DGE (tail
outs through SP, idle by then) so they never head-of-line-block the
Y in-DMAs.
Spikes return as uint8 and are unpacked/unpermuted to f32 on host.
"""

import numpy as np

import concourse.bacc as bacc
import concourse.mybir as mybir
from concourse import tile
from concourse.bass_utils import run_bass_kernel_spmd

B, T, N = 64, 100, 4096
NCORES = 8
NPC = N // NCORES         # 512 features per core
G = 4                     # quartet: features per partition
P = NPC // G              # 128 partitions
F = G * B                 # 256 free elems (quartet member x batch)
H = F // 2                # half-width for the interleaved chain streams

# pipeline structure (tuned against the TimelineSim cost model)
CHUNKS = [3, 3, 4] + [10] * 8 + [8, 2]          # Y in-DMA chunking
SPIKE_GROUPS = [(i * 10, 10, "pool") for i in range(8)] + [
    (80, 5, "pool"),
    (85, 5, "pool"),
    (90, 8, "split"),   # cols 0:C8 on Pool (free by then), rest on DVE
    (98, 2, "dve"),
]
C8 = 66                 # Pool's column share of the (90,8) spike group
XBUFS = {10: 6, 8: 3, 4: 4, 3: 3, 2: 3}
SBUFS = 8

_F32 = mybir.dt.float32
_I16 = mybir.dt.int16
_U8 = mybir.dt.uint8
_ALU = mybir.AluOpType

# ---------------------------------------------------------------- custom op --

_LIF_OP = None


def _register_lif_op():
    """Register the fused step op imm2*select(w<0, w, s0) + in1 at runtime."""
    global _LIF_OP
    if _LIF_OP is not None:
        return _LIF_OP
    from concourse.dve_spec import C0, C2, Spec, Src0, Src1, Zero, select, lower
    from concourse.dve_uop import DveOpSpec
    from concourse import dve_ops as dom

    name = "LIF_STEP_ANT"
    for op in dom.OPS:
        if op.name == name:
            _LIF_OP = op
            return op

    spec = Spec(
        body=C2 * select(Src0 < Zero, Src0, C0) + Src1,
        reference=lambda in0, in1, s0, s1, imm2: (
            np.float32(imm2) * np.where(in0 < 0, in0, s0).astype(np.float32) + in1
        ).astype(np.float32),
    )
    shas = {}
    for ver in ("v3", "v4"):
        try:
            tmp = DveOpSpec(name=name, opcode=None, uops=lower(spec, ver=ver), rd1_en=True)
            shas[ver] = tmp.sha(ver)
        except Exception:
            pass
    op = dom.DveOp(name, spec, subdim=False, uops_sha=shas)
    dom.OPS.append(op)
    dom._SUB_OPCODE_FOR_NAME[name] = dom._CUSTOM_DVE_ROW_BASE + len(dom.OPS) - 1
    dom.CUSTOM_DVE_SPECS[name] = spec
    _LIF_OP = op
    return op


# ------------------------------------------------------------------ program --

_NC_CACHE = {}


def _build_bass():
    if "nc" in _NC_CACHE:
        return _NC_CACHE["nc"]
    from collections import Counter

    lif_op = _register_lif_op()
    ccount = Counter(CHUNKS)
    gcount = Counter(gl for _, gl, _ in SPIKE_GROUPS)

    nc = bacc.Bacc("TRN2", name="lif_kernel")
    yt = nc.dram_tensor("yt", [P, T, F], _I16, kind="ExternalInput")
    nth = nc.dram_tensor("nth", [P, 2], _F32, kind="ExternalInput")
    spk = nc.dram_tensor("spk", [P, T, F], _U8, kind="ExternalOutput")

    with tile.TileContext(nc) as tc:
        with (
            tc.tile_pool(name="const", bufs=1) as cpool,
            tc.tile_pool(name="xin", bufs=1) as xpool,
            tc.tile_pool(name="sout", bufs=1) as spool,
        ):
            # nth col 0 = -th_mid/scale (reset arm), col 1 = -0.5*th_mid/scale
            nth_t = cpool.tile([P, 2], _F32)
            # w history: slot t+1 holds W after step t (slot 0 unused)
            w_all = cpool.tile([P, T + 1, F], _F32)

            gi = 0
            first = True
            t0c = 0
            for tchk in CHUNKS:
                x_tile = xpool.tile(
                    [P, tchk, F], _I16, tag=f"x{tchk}",
                    bufs=min(XBUFS[tchk], ccount[tchk]),
                )
                nc.sync.dma_start(x_tile[:], yt[:, t0c:t0c + tchk, :])
                if first:
                    # nth rides Pool's SWDGE so its descriptor gen never
                    # occupies the shared HWDGE slot the x chunks need
                    first = False
                    nc.gpsimd.dma_start(nth_t[:], nth[:])
                for tl in range(tchk):
                    t = t0c + tl
                    if t == 0:
                        # W0 = 0 makes the select trivial: 2x_2p tensor_scalar
                        # ops replace the two custom ops; issued as halves so
                        # the successors' sem round-trips overlap execution
                        for a, b in ((0, H), (H, F)):
                            nc.vector.tensor_scalar(
                                out=w_all[:, 1, a:b], in0=x_tile[:, 0, a:b],
                                scalar1=1.0, scalar2=nth_t[:, 1:2],
                                op0=_ALU.mult, op1=_ALU.add,
                            )
                        continue
                    nc.vector._custom_dve(
                        lif_op, out=w_all[:, t + 1, 0:H], in0=w_all[:, t, 0:H],
                        in1=x_tile[:, tl, 0:H], s0=nth_t[:, 0:1], imm2=0.5,
                    )
                    nc.vector._custom_dve(
                        lif_op, out=w_all[:, t + 1, H:F], in0=w_all[:, t, H:F],
                        in1=x_tile[:, tl, H:F], s0=nth_t[:, 0:1], imm2=0.5,
                    )
                t0c += tchk
                while gi < len(SPIKE_GROUPS) and SPIKE_GROUPS[gi][0] + SPIKE_GROUPS[gi][1] <= t0c:
                    g0, gl, eng = SPIKE_GROUPS[gi]
                    s_tile = spool.tile(
                        [P, gl, F], _U8, tag=f"s{gl}",
                        bufs=min((SBUFS if gl >= 10 else 3), gcount[gl]),
                    )
                    wsl = w_all[:, g0 + 1:g0 + gl + 1, :]
                    if eng == "dve":
                        if gl == 2:
                            # final group: halves hide the post-chain sem gap
                            for a, b in ((0, H), (H, F)):
                                nc.vector.tensor_scalar(
                                    out=s_tile[:, :, a:b],
                                    in0=w_all[:, g0 + 1:g0 + gl + 1, a:b],
                                    scalar1=0.0, scalar2=None, op0=_ALU.is_ge,
                                )
                        else:
                            nc.vector.tensor_scalar(
                                out=s_tile[:], in0=wsl, scalar1=0.0, scalar2=None,
                                op0=_ALU.is_ge,
                            )
                        # tail outs ride the (idle by now) SP queue
                        nc.sync.dma_start(spk[:, g0:g0 + gl, :], s_tile[:])
                    elif eng == "split":
                        # asymmetric split: Pool (idle, slow) takes C8 cols in
                        # parallel with the chain; DVE takes the rest so its
                        # stream insert stays short — balances the two
                        # out-DMA readiness paths at the tail
                        nc.gpsimd.tensor_scalar(
                            out=s_tile[:, :, 0:C8],
                            in0=w_all[:, g0 + 1:g0 + gl + 1, 0:C8],
                            scalar1=0.0, scalar2=None, op0=_ALU.is_ge,
                        )
                        nc.vector.tensor_scalar(
                            out=s_tile[:, :, C8:F],
                            in0=w_all[:, g0 + 1:g0 + gl + 1, C8:F],
                            scalar1=0.0, scalar2=None, op0=_ALU.is_ge,
                        )
                        nc.sync.dma_start(spk[:, g0:g0 + gl, :], s_tile[:])
                    else:
                        nc.gpsimd.tensor_scalar(
                            out=s_tile[:], in0=wsl, scalar1=0.0, scalar2=None,
                            op0=_ALU.is_ge,
                        )
                        nc.scalar.dma_start(spk[:, g0:g0 + gl, :], s_tile[:])
                    gi += 1

    nc.finalize()
    _NC_CACHE["nc"] = nc
    return nc


# -------------------------------------------------------------------- entry --

def _prep_inputs(x, thresh):
    """Host-side permute/shift/quantize: returns (in_maps, order)."""
    x = np.ascontiguousarray(x, dtype=np.float32)
    thresh = np.ascontiguousarray(thresh, dtype=np.float32)
    order = np.argsort(thresh, kind="stable")
    in_maps = []
    for c in range(NCORES):
        feat = order[c * NPC:(c + 1) * NPC]
        th_c = thresh[feat]                           # sorted within core
        quart = th_c.reshape(P, G)
        th_mid = (quart[:, 0] + quart[:, -1]) * np.float32(0.5)
        ys = x[:, :, feat] - np.float32(0.5) * th_c[None, None, :]
        scale = np.float32(max(np.abs(ys).max() / 32000.0, 1e-30))
        yq = np.round(ys / scale).astype(np.int16)
        nth0 = (-th_mid / scale).astype(np.float32)
        nth = np.ascontiguousarray(
            np.stack([nth0, np.float32(0.5) * nth0], axis=1)
        )
        # [b, t, p*G+j] -> [p, t, j*B+b]
        yc = (
            yq.reshape(B, T, P, G)
            .transpose(2, 1, 3, 0)
            .reshape(P, T, F)
        )
        in_maps.append({"yt": np.ascontiguousarray(yc), "nth": nth})
    return in_maps, order


def _run(x, thresh, trace=False):
    nc = _build_bass()
    in_maps, order = _prep_inputs(x, thresh)
    res = run_bass_kernel_spmd(
        nc, in_maps, core_ids=list(range(NCORES)), trace=trace
    )
    out = np.empty((B, T, N), dtype=np.float32)
    for c in range(NCORES):
        feat = order[c * NPC:(c + 1) * NPC]
        s = np.asarray(res.results[c]["spk"])          # [P, T, F] u8
        vals = (
            s.reshape(P, T, G, B)
            .transpose(3, 1, 0, 2)
            .reshape(B, T, NPC)
        )
        out[:, :, feat] = vals.astype(np.float32)
    return out, res


def kernel(x, thresh):
    out, _ = _run(x, thresh, trace=False)
    return out
